# revision 42
# baseline (speedup 1.0000x reference)
"""Trainium2 Bass kernel for C = triu(A @ B), A/B upper-triangular 4096x4096 f32.

kernel(**inputs) takes FULL inputs {"A","B"} and returns the FULL output,
sharding across 8 NeuronCores via run_bass_kernel_spmd (SPMD: one program,
per-core data).

Design (v2, banded quad sweeps):
  C tiled into 128x512 supers (bi=row-block 0..31, jg=col-group 0..7);
  super (bi,jg) needs contraction over bk in [bi, 4jg+3]. The bk axis of
  each column jg is cut into LEFT-ALIGNED bands of 8 (last band is 4 for
  even jg). Work unit = "quad sweep": 4 supers with consecutive bi
  accumulate in 4 PSUM banks over ONE shared B stream covering a band.
  Partial results per (quad, band) are summed on the host.

  Quad types (all template-static):
    F8M: full engagement, 8 steps, all-512 widths        (16 globally)
    F8D: full, 8 steps, diagonal widths [..,384,256,128] (12)
    S8M: staircase (track r engages at step r), 8 steps  (12)
    S8D: staircase, 8 steps, diagonal widths             (4)
    S4M: staircase, 4 steps, all-512                     (12)
    S4D: staircase, 4 steps, widths [512,384,256,128]    (4)
    F4D: full, 4 steps, widths [512,384,256,128]         (12)
  Per-core template (9 slots, identical on all cores):
    2xF8M, 2xF8D (4 of them host F4D right-aligned), 2xS8M (4 host S8D),
    2xS4M (4 host S4D), 1xF4D.
  Padding is done with zero stationary blocks packed inline; all cores
  execute identical instruction streams => perfect balance.

  Per-core packed input:  S [steps, 128, 1024] f32; per step: 4 A^T blocks
  (cols 0:512) + B block (cols 512:1024). One fused DMA per step.
  Output: CP [36, 128, 512] partials; host accumulates into C.

  MODE: "split3" (default: 3x bf16 hi/lo matmuls, 3 cyc/row, rel err ~4.5e-6
  -- inside the fp32 accumulation-reordering envelope), "fp32" (exact,
  4 cyc/row), or "f32r" (tf32-class, ~1.5e-4).
"""

import sys

sys.path.insert(0, "/opt/trn_rl_repo")

import numpy as np

N = 4096
N_CORES = 8
NB = N // 128
NJ = N // 512

MODE = "split3"

# ---------------------------------------------------------------- schedule


def _enumerate_quads():
    """All real quads: (type, jg, band_a, band_b, r0).

    band [a,b] bk-range; quad rows bi in [r0, r0+3].
    """
    quads = []
    for jg in range(NJ):
        R = 4 * jg + 4
        # left-aligned bands of 8; trailing 4-band when R % 8 == 4
        bands = []
        a = 0
        while a < R:
            blen = 8 if R - a >= 8 else 4
            bands.append((a, a + blen - 1))
            a += blen
        for (a, b) in bands:
            diag = (b == R - 1)
            blen = b - a + 1
            # full quads: rows strictly above band
            for t in range(a // 4):
                if blen == 8:
                    quads.append(("F8D" if diag else "F8M", jg, a, b, 4 * t))
                else:
                    assert diag
                    quads.append(("F4D", jg, a, b, 4 * t))
            # staircase quads: rows inside the band
            if blen == 8:
                quads.append(("S8D" if diag else "S8M", jg, a, b, a))
                # lower staircase = 4-step stair over the band's last 4 bks
                quads.append(("S4D" if diag else "S4M", jg, a + 4, b, a + 4))
            else:
                quads.append(("S4D" if diag else "S4M", jg, a, b, a))
    return quads


# template slot types -> (steps, engagement, widths)
def _tmpl(ttype):
    L = 8 if "8" in ttype else 4
    stair = ttype.startswith("S")
    diag = ttype.endswith("D")
    widths = []
    for t in range(L):
        rem = L - 1 - t
        w = 512
        if diag and rem < 3:
            w = 128 * (rem + 1)
        widths.append(w)
    engage = [(0 if not stair else r) for r in range(4)]
    return L, engage, widths


# per-core slot list: (template_type, hosted types allowed). Order matters
# for overlap: medium slot first (covers the DMA fill), small slots in the
# middle, the two big F8M slots last so their dense matmul streams cover the
# preceding evictions and shorten the tail.
_SLOT_TYPES = ["F8D", "S4M", "S8M", "S4D", "F8M", "F4D", "S8M", "S4M",
               "F8D", "F8M"]


def _build_assignment():
    """assign[core][slot] = quad (type, jg, a, b, r0) hosted there."""
    quads = _enumerate_quads()
    by_type = {}
    for q in quads:
        by_type.setdefault(q[0], []).append(q)
    for t in by_type:
        by_type[t].sort(key=lambda q: (q[1], q[2], q[4]))
    counts = {t: len(v) for t, v in by_type.items()}
    assert counts == {"F8M": 16, "F8D": 12, "F4D": 12, "S8M": 12,
                      "S8D": 4, "S4M": 12, "S4D": 8}, counts
    # per-template-type host queues; each slot of that type pops 8 quads.
    # F8D templates host F8D + F4D (right-aligned); S8M host S8M + S8D;
    # S4M host S4M + S4D; the dedicated S4D slot takes the rest + 4 ghosts.
    queues = {
        "F8M": by_type["F8M"],
        "F8D": by_type["F8D"] + by_type["F4D"][:4],
        "S8M": by_type["S8M"] + by_type["S8D"],
        "S4M": by_type["S4M"] + by_type["S4D"][:4],
        "F4D": by_type["F4D"][4:],
        "S4D": by_type["S4D"][4:] + [None] * 4,
    }
    pos = {t: 0 for t in queues}
    assign = [[None] * len(_SLOT_TYPES) for _ in range(N_CORES)]
    for s, ttype in enumerate(_SLOT_TYPES):
        q = queues[ttype]
        grp = q[pos[ttype]:pos[ttype] + 8]
        pos[ttype] += 8
        assert len(grp) == 8, (s, ttype, len(grp))
        for c in range(N_CORES):
            assign[c][s] = grp[c]
    for t in queues:
        assert pos[t] == len(queues[t]), (t, pos[t], len(queues[t]))
    return assign


_ASSIGN = _build_assignment()
_TOTAL_STEPS = sum(_tmpl(t)[0] for t in _SLOT_TYPES)  # 60

_cache = {}


def _eff_w(w):
    if MODE == "f32r" and w < 256:
        return 256  # f32r runs at 1/4 rate below 256 cols
    return w


def _layout():
    """Variable-width per-step stream layout (template-static).

    Per step only the engaged A tracks and the live B columns are shipped.
    Element layout (au = A track unit cols, bu = B units):
      [A track 0 .. A track e-1 | B unit 0 (w cols) .. B unit bu-1]
    split3: au=256 (Ah|Al), bu=2 (Bh,Bl), bf16. fp32/f32r: au=128, bu=1, f32.
    Returns (steps, total_words): steps[i] = (e, w, au, bu, word_ofs, wpp).
    """
    au = 256 if MODE == "split3" else 128
    bu = 2 if MODE == "split3" else 1
    steps = []
    ofs = 0
    for ttype in _SLOT_TYPES:
        L, engage, widths = _tmpl(ttype)
        for t in range(L):
            e = sum(1 for r in range(4) if t >= engage[r])
            w = _eff_w(widths[t])
            wpp = au * e + bu * w
            steps.append((e, w, au, bu, ofs, wpp))
            ofs += 128 * wpp
    return steps, ofs

# ------------------------------------------------------------------ device


def _build_nc():
    import concourse.bacc as bacc
    import concourse.mybir as mybir
    import concourse.tile as tile

    f32 = mybir.dt.float32
    nc = bacc.Bacc()
    if MODE == "split3":
        s_dt = mybir.dt.bfloat16
        store_dt = mybir.dt.bfloat16
    else:
        s_dt = {"fp32": mybir.dt.float32, "f32r": mybir.dt.float32r}[MODE]
        store_dt = mybir.dt.float32
    steps_layout, total_words = _layout()
    s_in = nc.declare_dram_parameter("S", [total_words], store_dt,
                                     isOutput=False)
    cp = nc.declare_dram_parameter("CP", [4 * len(_SLOT_TYPES), 128, 512], f32,
                                   isOutput=True)

    with tile.TileContext(nc) as tc:
        with (
            tc.tile_pool(name="st", bufs=16) as s_pool,
            tc.tile_pool(name="co", bufs=12) as c_pool,
            tc.tile_pool(name="ps", bufs=2, space="PSUM") as ps_pool,
        ):
            cursor = 0
            for s, ttype in enumerate(_SLOT_TYPES):
                L, engage, widths = _tmpl(ttype)
                ps = [
                    ps_pool.tile([128, 512], f32, tag=f"p{r}",
                                 name=f"ps_{s}_{r}")
                    for r in range(4)
                ]
                for t in range(L):
                    e, w, au, bu, ofs, wpp = steps_layout[cursor]
                    oc = 512 - w
                    src = s_in[ofs:ofs + 128 * wpp] \
                        .rearrange("(p w) -> p w", p=128).bitcast(s_dt)
                    st = s_pool.tile([128, wpp], s_dt, tag="s",
                                     name=f"st_{s}_{t}")
                    half = wpp // 2
                    nc.sync.dma_start(out=st[:, :half], in_=src[:, :half])
                    nc.gpsimd.dma_start(out=st[:, half:], in_=src[:, half:])
                    for r in range(4):
                        if t < engage[r]:
                            continue
                        first = (t == engage[r])
                        last = (t == L - 1)
                        if MODE == "split3":
                            ah = st[:, au * r:au * r + 128]
                            al = st[:, au * r + 128:au * (r + 1)]
                            bh = st[:, au * e:au * e + w]
                            bl = st[:, au * e + w:au * e + 2 * w]
                            nc.tensor.matmul(ps[r][:, oc:], lhsT=ah, rhs=bh,
                                             start=first, stop=False)
                            nc.tensor.matmul(ps[r][:, oc:], lhsT=al, rhs=bh,
                                             start=False, stop=False)
                            nc.tensor.matmul(ps[r][:, oc:], lhsT=ah, rhs=bl,
                                             start=False, stop=last)
                        else:
                            nc.tensor.matmul(
                                ps[r][:, oc:],
                                lhsT=st[:, au * r:au * (r + 1)],
                                rhs=st[:, au * e:au * e + w],
                                start=first, stop=last,
                            )
                    cursor += 1
                for r in range(4):
                    c_t = c_pool.tile([128, 512], f32, tag="c",
                                      name=f"c_{s}_{r}")
                    nc.vector.tensor_copy(c_t[:], ps[r][:])
                    nc.sync.dma_start(out=cp[4 * s + r], in_=c_t[:])
            assert cursor == _TOTAL_STEPS
    nc.finalize()
    return nc


def get_nc():
    key = ("nc", MODE)
    if key not in _cache:
        _cache[key] = _build_nc()
    return _cache[key]


# ------------------------------------------------------------------- host


def _make_blocks(A, B):
    """Mode-specific block views for packing."""
    A4 = A.reshape(NB, 128, NB, 128).transpose(0, 2, 3, 1)
    B4 = B.reshape(NB, 128, NJ, 512).transpose(0, 2, 1, 3)
    if MODE != "split3":
        return {"A": [A4], "B": [B4], "dtype": np.float32}
    import ml_dtypes

    bf = ml_dtypes.bfloat16
    A4h = A4.astype(bf)
    A4l = (A4 - A4h.astype(np.float32)).astype(bf)
    B4h = B4.astype(bf)
    B4l = (B4 - B4h.astype(np.float32)).astype(bf)
    return {"A": [A4h, A4l], "B": [B4h, B4l], "dtype": bf}


def _pack_core(c, blocks):
    """Flat variable-width S stream for core c (layout per _layout()).

    A blocks are transposed ([p,m] = A[128bi+m, 128bk+p]); B blocks are
    128x512 (only the live [oc:] columns are shipped).
    """
    steps_layout, total_words = _layout()
    S = np.zeros(total_words, dtype=blocks["dtype"])
    cursor = 0
    for s, ttype in enumerate(_SLOT_TYPES):
        L, engage, widths = _tmpl(ttype)
        q = _ASSIGN[c][s]
        if q is None:  # ghost slot: leave zeros
            cursor += L
            continue
        qtype, jg, a, b, r0 = q
        base = b - L + 1  # bk at template step 0 (right-aligned hosting)
        for t in range(L):
            e, w, au, bu, ofs, wpp = steps_layout[cursor]
            bk = base + t
            row = S[ofs:ofs + 128 * wpp].reshape(128, wpp)
            oc = 512 - w
            if bk >= a:
                for h in range(bu):
                    row[:, au * e + w * h:au * e + w * (h + 1)] = \
                        blocks["B"][h][bk, jg][:, oc:]
            for r in range(e):
                bi = r0 + r
                if bk >= a and bk >= bi:
                    for h in range(len(blocks["A"])):
                        row[:, au * r + 128 * h:au * r + 128 * (h + 1)] = \
                            blocks["A"][h][bi, bk]
            cursor += 1
    return S


def _get_runner():
    """Build (once per process/MODE) a cached jitted SPMD executable.

    Mirrors bass2jax.run_bass_via_pjrt's multi-core path, but reuses the
    compiled executable across kernel() calls.
    """
    key = ("runner", MODE)
    if key in _cache:
        return _cache[key]
    import jax
    from jax.sharding import Mesh, PartitionSpec
    from jax.experimental.shard_map import shard_map
    from concourse import bass2jax, mybir

    nc = get_nc()
    bass2jax.install_neuronx_cc_hook()
    partition_name = (nc.partition_id_tensor.name
                      if nc.partition_id_tensor else None)
    out_shape = (4 * len(_SLOT_TYPES), 128, 512)
    out_aval = jax.core.ShapedArray(out_shape, np.float32)
    in_names = ["S", "CP"]
    if partition_name is not None:
        in_names.append(partition_name)

    def _body(s_arr, zeros):
        operands = [s_arr, zeros]
        if partition_name is not None:
            operands.append(bass2jax.partition_id_tensor())
        outs = bass2jax._bass_exec_p.bind(
            *operands, out_avals=(out_aval,), in_names=tuple(in_names),
            out_names=("CP",), lowering_input_output_aliases=(),
            sim_require_finite=True, sim_require_nnan=True, nc=nc)
        return outs[0]

    devices = jax.devices()[:N_CORES]
    mesh = Mesh(np.asarray(devices), ("core",))
    sharded = jax.jit(
        shard_map(_body, mesh=mesh,
                  in_specs=(PartitionSpec("core"),) * 2,
                  out_specs=PartitionSpec("core"), check_rep=False),
        donate_argnums=(1,), keep_unused=True)
    _cache[key] = sharded
    return sharded


def kernel(A: np.ndarray, B: np.ndarray) -> np.ndarray:
    A = np.asarray(A, dtype=np.float32)
    B = np.asarray(B, dtype=np.float32)

    blocks = _make_blocks(A, B)
    s_all = np.concatenate([_pack_core(c, blocks) for c in range(N_CORES)],
                           axis=0)
    zeros = np.zeros((N_CORES * 4 * len(_SLOT_TYPES), 128, 512), np.float32)
    runner = _get_runner()
    out = np.asarray(runner(s_all, zeros))
    per_core = out.reshape(N_CORES, 4 * len(_SLOT_TYPES), 128, 512)

    C = np.zeros((N, N), dtype=np.float32)
    for c in range(N_CORES):
        cpk = per_core[c]
        for s, ttype in enumerate(_SLOT_TYPES):
            q = _ASSIGN[c][s]
            if q is None:
                continue
            qtype, jg, a, b, r0 = q
            for r in range(4):
                bi = r0 + r
                blk = cpk[4 * s + r]
                # written psum region starts at the track's start width
                L, engage, widths = _tmpl(ttype)
                w0 = widths[engage[r]]
                if MODE == "f32r" and w0 < 256:
                    w0 = 256
                lo = 512 - w0
                C[128 * bi:128 * (bi + 1),
                  512 * jg + lo:512 * (jg + 1)] += blk[:, lo:]
    return C


def _make_in_maps(A, B):
    A = np.asarray(A, dtype=np.float32)
    B = np.asarray(B, dtype=np.float32)
    blocks = _make_blocks(A, B)
    return [{"S": _pack_core(c, blocks)} for c in range(N_CORES)]
